# revision 25
# baseline (speedup 1.0000x reference)
"""Trainium2 Bass kernel for local windowed per-channel attention (sparse_attention).

Reference computation (per batch b, channel c, position (h,w)):
    q = W_q x ; k = W_k x_pad ; v = W_v x_pad           (1x1 convs)
    s[i,j]  = q[h,w] * (k[h+i, w+j] + bias[c, i or j])  over a 7x7 window
    out     = sum_ij softmax_ij(s) * v[h+i, w+j]

Sharding: spatial, 8 ways — core = (batch, 12-row slab). Fully independent
per core (no collectives). Host pre-pads each slab with the 3-row/col halo.

Per-core dataflow (channels on partitions, 2 channel-tiles of 128):
  TensorE : q/k/v GEMMs on fp16 inputs; 49-tap reduction of den/num via
            identity-matmul accumulation into PSUM — [e|m] interleaved per
            tap so den|num share one 3-bank region. The 7 tap-slots of a
            group accumulate in ONE matmul via a slot-spanning rhs AP and a
            stride-0 PSUM out AP (PSUM accumulate), cutting PE instruction
            count ~7x (PE SEQ decode was a bottleneck).
  VectorE : score mult q*kb and weight mult e*v in fp16/bf16 at the DVE 2x
            mode via shifted-window access patterns. Per-(c,tap) rel-pos
            bias pre-folded into 7 biased fp16 k maps with 4x-mode
            tensor_scalar adds; odd-kw groups read 1-col-shifted copies
            (kb1/v1) so every innermost fp16 run stays 4B-aligned.
  ScalarE : exp in place on the e slots (fp16 scores -> bf16), plus k/q
            PSUM evictions (v evictions go to VectorE).
  GpSimd  : a tuned slice of the weight-mult halves.
  DMA     : 3 issue queues (SP/ACT/DVE) so HWDGE descriptor generation
            (~625ns per dma_start) overlaps; x and the k-part of the
            weights land first so the k GEMM starts ~1.5us in.
  Front   : GEMMs run in 1-bank PSUM chunk tiles that coexist with the
            attention PSUM region (6+2 = 8 banks), so the first attention
            groups overlap the remaining GEMMs.
  out = num / den  (reciprocal + mult), f16 out DMA, host upcast.
"""
import os
import numpy as np

from concourse import bass, bacc, mybir, tile
from concourse.bass_utils import run_bass_kernel_spmd

F32 = mybir.dt.float32
F16 = mybir.dt.float16
BF16 = mybir.dt.bfloat16

K, PAD = 7, 3
B, CIN, COUT, H, W = 2, 256, 256, 48, 48
ROWS = 12                 # output rows per core
SH, SW = ROWS + 2 * PAD, W + 2 * PAD   # 18, 54 padded slab
NPOS = ROWS * W           # 576 output positions per core
NPAD = SH * SW            # 972 padded positions
N_CORES = 8

SCORE_DT = F16            # kb, q, s dtype (f16 -> DVE 2x mode)
E_DT = BF16               # exp output / matmul rhs dtype (needs bf16 range)
V_DT = F16
GEMM_DT = os.environ.get("GEMM_DT", "f16")
TAPW = 2 * NPOS           # per-tap [e | m] interleaved width
RING_BUFS = int(os.environ.get("RING_BUFS", "7"))
WARM = int(os.environ.get("WARM", "32"))

GORD = [0, 2, 4, 6, 1, 3, 5]
# m-mult halves handed to GpSimd: per-tile GORD position -> half
# (half 0 = slots 0:4, half 1 = slots 4:7)
_PP = os.environ.get("POOLPLAN", "0:1,1:1,2:1,3:1,4:1")
POOL_PLAN = {int(p.split(":")[0]): int(p.split(":")[1])
             for p in _PP.split(",") if p}

_CACHED = {}


def _fap(t, offset, dims):
    """Custom free-dim AP on a tile: dims = [[stride, size], ...]."""
    a = t[:]
    return bass.AP(a.tensor, a.offset + offset, [list(a.ap[0])] + dims)


def _emit_body(nc, tc, dram):
    x_d, w_d, cst_d, out_d = dram
    MULT = mybir.AluOpType.mult
    ESPLIT = int(os.environ.get("ESPLIT", "3"))
    SSPLIT = int(os.environ.get("SSPLIT", "2"))
    with (
        tc.tile_pool(name="const", bufs=1) as const,
        tc.tile_pool(name="work", bufs=1) as work,
    ):
        GDT = {"f16": F16, "f32": F32}[GEMM_DT]
        # ---- input DMAs on 3 queues ----
        cst = const.tile([128, 2 * K + 128], F32, name="cst")
        wt_sb = [const.tile([128, 3 * COUT], GDT, name=f"w{kt}")
                 for kt in range(2)]
        x_sb = [const.tile([128, NPAD], GDT, name=f"x_sb{kt}")
                for kt in range(2)]
        # junk tile, ready ~instantly: feeds the PE warm-up matmuls and the
        # exp-table warm activation so neither waits on an input DMA
        junk = const.tile([128, 128], E_DT, name="junk")
        nc.gpsimd.memset(junk[:], 0.0)
        XS = 512
        # Three DMA queues. SP ends with only the out DMAs queued behind it
        # (in-order queues: a dependency-stalled copy blocks everything
        # later, so the SBUF shift copies live on the gpsimd/SWDGE queue).
        # tile1 weight slices first: attention starts with tile1, whose
        # groups need only one kb slab each.
        nc.sync.dma_start(x_sb[0][:, :XS], x_d[0:128, :XS])
        nc.sync.dma_start(wt_sb[1][:, :COUT], w_d[128:256, :COUT])
        nc.sync.dma_start(cst[:], cst_d[:, :])
        nc.sync.dma_start(wt_sb[1][:, COUT:], w_d[128:256, COUT:])
        nc.sync.dma_start(wt_sb[0][:, :COUT], w_d[0:128, :COUT])
        nc.sync.dma_start(wt_sb[0][:, COUT:], w_d[0:128, COUT:])
        nc.scalar.dma_start(x_sb[1][:, :XS], x_d[128:256, :XS])
        nc.gpsimd.dma_start(x_sb[0][:, XS:], x_d[0:128, XS:])
        nc.gpsimd.dma_start(x_sb[1][:, XS:], x_d[128:256, XS:])

        beta_sb = cst[:, :2 * K]
        idf = cst[:, 2 * K:]
        idb = const.tile([128, 128], E_DT, name="idb")
        nc.vector.tensor_copy(idb[:], idf)
        # warm the ACT exp table while DMAs stream in
        warm = const.tile([128, 2], F32, name="warm")
        nc.scalar.activation(warm[:], junk[:, :4].bitcast(F32),
                             mybir.ActivationFunctionType.Exp)
        base = {"k": 0, "q": COUT, "v": 2 * COUT}
        w_sb = {(nm, kt): wt_sb[kt][:, base[nm]:base[nm] + COUT]
                for nm in base for kt in range(2)}

        # ---- per channel-tile persistent tensors ----
        k0 = [work.tile([128, NPAD], SCORE_DT, name=f"k0_{mt}")
              for mt in range(2)]
        kb0 = [work.tile([128, K * NPAD], SCORE_DT, name=f"kb0_{mt}")
               for mt in range(2)]
        kb1 = [work.tile([128, K * NPAD], SCORE_DT, name=f"kb1_{mt}")
               for mt in range(2)]
        v0 = [work.tile([128, NPAD], V_DT, name=f"v0_{mt}") for mt in range(2)]
        v1 = [work.tile([128, NPAD], V_DT, name=f"v1_{mt}") for mt in range(2)]
        q_sb = [work.tile([128, NPOS], SCORE_DT, name=f"q{mt}")
                for mt in range(2)]

        order = [(g, mt) for mt in (1, 0) for g in GORD]
        pool_set = set()
        for mt in range(2):
            for pos, half in POOL_PLAN.items():
                pool_set.add((GORD[pos], mt, half))

        with (
            tc.tile_pool(name="apsum", bufs=1, space="PSUM") as apsum,
            tc.tile_pool(name="gpsum", bufs=2, space="PSUM") as gpsum,
            tc.tile_pool(name="ring", bufs=RING_BUFS) as ring,
        ):
            dn_ps = [apsum.tile([128, TAPW], F32, name=f"dn{mt}")
                     for mt in range(2)]

            # HAM warm-up: dummy matmuls during the input-DMA wait so the
            # PE clock-gate opens before the real GEMM burst.
            wps = gpsum.tile([128, 512], F32, tag="gp", name="wps")
            for wi in range(WARM):
                nc.tensor.matmul(wps[:, :128], junk[:], junk[:],
                                 start=(wi == 0), stop=(wi == WARM - 1))

            # ---- GEMM emitters (1-bank chunked PSUM + evictions) ----
            def kv_gemm(nm, mt, dst, evict):
                mm = slice(mt * 128, (mt + 1) * 128)
                for c0, c1 in ((0, 512), (512, NPAD)):
                    p = gpsum.tile([128, 512], F32, tag="gp",
                                   name=f"{nm}p{mt}_{c0}")
                    for kt in range(2):
                        nc.tensor.matmul(p[:, :c1 - c0], w_sb[nm, kt][:, mm],
                                         x_sb[kt][:, c0:c1],
                                         start=(kt == 0), stop=(kt == 1))
                    evict(dst[mt][:, c0:c1], p[:, :c1 - c0])

            def q_gemm(mt):
                mm = slice(mt * 128, (mt + 1) * 128)
                for r0, nr in ((0, 6), (6, 6)):
                    p = gpsum.tile([128, 512], F32, tag="gp",
                                   name=f"qp{mt}_{r0}")
                    for kt in range(2):
                        rhs = _fap(x_sb[kt], (PAD + r0) * SW + PAD,
                                   [[SW, nr], [1, W]])
                        nc.tensor.matmul(p[:, :nr * W], w_sb["q", kt][:, mm],
                                         rhs, start=(kt == 0), stop=(kt == 1))
                    nc.scalar.copy(q_sb[mt][:, r0 * W:(r0 + nr) * W],
                                   p[:, :nr * W])

            def bias_adds(mt, slabs):
                for t in slabs:
                    nc.vector.tensor_scalar_add(
                        kb0[mt][:, t * NPAD:(t + 1) * NPAD], k0[mt][:],
                        beta_sb[:, mt * K + t:mt * K + t + 1])

            def kb1_copies(mt):
                # strided multi-slab 1-col-shifted copies on the SWDGE queue
                if mt == 0:
                    nc.gpsimd.dma_start(
                        _fap(kb1[0], 0, [[NPAD, K], [1, NPAD - 2]]),
                        _fap(kb0[0], 1, [[NPAD, K], [1, NPAD - 2]]))
                else:
                    # tile1's odd groups read only kb1 slabs 1, 3, 5
                    nc.gpsimd.dma_start(
                        _fap(kb1[1], NPAD, [[2 * NPAD, 3], [1, NPAD - 2]]),
                        _fap(kb0[1], NPAD + 1, [[2 * NPAD, 3], [1, NPAD - 2]]))

            # ---- attention stage emitters ----
            em_tiles = {}
            esplit = set(order[:ESPLIT])
            esplit.add(order[-1])
            ssplit = set(order[:SSPLIT])

            EM_BUFS = {1: RING_BUFS // 2, 0: RING_BUFS - RING_BUFS // 2}

            def emit_se(g, mt):
                em_t = ring.tile([128, K * TAPW], E_DT, tag=f"em{mt}",
                                 bufs=EM_BUFS[mt], name=f"em{mt}_{g}")
                em_tiles[(g, mt)] = em_t
                par = g % 2
                # Both tiles: groups = kw (j), slots = kh (i). Tile0's bias
                # varies with i == slot (slab advances with slot, stride
                # NPAD+SW); tile1's bias slab is g (stride SW).
                if mt == 0:
                    kb_ap = _fap(kb1[0] if par else kb0[0], (g - par),
                                 [[NPAD + SW, K], [SW, ROWS], [1, W]])
                else:
                    kb_ap = _fap(kb1[1] if par else kb0[1],
                                 g * NPAD + (g - par),
                                 [[SW, K], [SW, ROWS], [1, W]])
                if (g, mt) in ssplit:
                    # first groups: score+exp in slot halves so the first
                    # weight-mults start one half earlier
                    for i0, ni in ((0, 4), (4, 3)):
                        q_ap = _fap(q_sb[mt], 0, [[0, ni], [W, ROWS], [1, W]])
                        s_ap = _fap(em_t, i0 * TAPW,
                                    [[TAPW, ni], [W, ROWS], [1, W]]
                                    ).bitcast(SCORE_DT)
                        kb_h = bass.AP(
                            kb_ap.tensor, kb_ap.offset + i0 * kb_ap.ap[1][0],
                            [list(kb_ap.ap[0]), [kb_ap.ap[1][0], ni]] +
                            [list(d) for d in kb_ap.ap[2:]])
                        nc.vector.tensor_tensor(s_ap, kb_h, q_ap, MULT)
                        nc.scalar.activation(
                            _fap(em_t, i0 * TAPW, [[TAPW, ni], [1, NPOS]]),
                            _fap(em_t, i0 * TAPW, [[TAPW, ni], [1, NPOS]]
                                 ).bitcast(SCORE_DT),
                            mybir.ActivationFunctionType.Exp)
                    return
                q_ap = _fap(q_sb[mt], 0, [[0, K], [W, ROWS], [1, W]])
                s_ap = _fap(em_t, 0, [[TAPW, K], [W, ROWS], [1, W]]
                            ).bitcast(SCORE_DT)
                nc.vector.tensor_tensor(s_ap, kb_ap, q_ap, MULT)
                if (g, mt) in esplit:
                    for i0, ni in ((0, 4), (4, 3)):
                        nc.scalar.activation(
                            _fap(em_t, i0 * TAPW, [[TAPW, ni], [1, NPOS]]),
                            _fap(em_t, i0 * TAPW, [[TAPW, ni], [1, NPOS]]
                                 ).bitcast(SCORE_DT),
                            mybir.ActivationFunctionType.Exp)
                else:
                    nc.scalar.activation(
                        _fap(em_t, 0, [[TAPW, K], [1, NPOS]]),
                        _fap(em_t, 0, [[TAPW, K], [1, NPOS]]
                             ).bitcast(SCORE_DT),
                        mybir.ActivationFunctionType.Exp)

            def emit_mu(g, mt):
                em_t = em_tiles[(g, mt)]
                par = g % 2
                for half, (i0, ni) in enumerate(((0, 4), (4, 3))):
                    e_ap = _fap(em_t, i0 * TAPW, [[TAPW, ni], [W, ROWS],
                                                  [1, W]])
                    v_ap = _fap(v1[mt] if par else v0[mt], g - par + i0 * SW,
                                [[SW, ni], [SW, ROWS], [1, W]])
                    m_ap = _fap(em_t, i0 * TAPW + NPOS,
                                [[TAPW, ni], [W, ROWS], [1, W]])
                    eng = (nc.gpsimd if (g, mt, half) in pool_set
                           else nc.vector)
                    eng.tensor_tensor(m_ap, e_ap, v_ap, MULT)

            CH = ((0, 512), (512, 1024), (1024, TAPW))
            done = set()

            def acc(em_t, mt, c0, c1, first, stop=False):
                # per-slot accumulate matmuls (the ISA caps one matmul at
                # 512 output elements, so tap-slots cannot be merged)
                for sl in range(K):
                    nc.tensor.matmul(
                        dn_ps[mt][:, c0:c1], idb[:],
                        em_t[:, sl * TAPW + c0:sl * TAPW + c1],
                        start=(first and sl == 0), stop=(stop and sl == K - 1))

            def outmul(o_t, rden, mt, c0, c1):
                nc.vector.tensor_tensor(
                    o_t[:, c0:c1], dn_ps[mt][:, NPOS + c0:NPOS + c1],
                    rden[:, c0:c1], MULT)
                nc.sync.dma_start(out_d[mt * 128:(mt + 1) * 128, c0:c1],
                                  o_t[:, c0:c1])

            def emit_ac_e(g, mt):
                # e-only chunk (cols 0:512): depends only on exp(g), so it
                # is emitted ahead of the previous group's m-chunks to keep
                # the in-order PE queue from head-of-line blocking on m
                em_t = em_tiles[(g, mt)]
                # pstate primer: a tiny dependency-free matmul absorbs the
                # slow first-instruction-after-idle PE state
                pr = gpsum.tile([128, 512], F32, tag="gp",
                                name=f"pr{mt}_{g}")
                nc.tensor.matmul(pr[:, :128], junk[:], junk[:],
                                 start=True, stop=True)
                acc(em_t, mt, *CH[0], g == GORD[0])

            def emit_ac(g, mt):
                em_t = em_tiles.pop((g, mt))
                first = (g == GORD[0])
                done.add((g, mt))
                tile_done = all((gg, mt) in done for gg in GORD)
                final = tile_done and (g, mt) == order[-1]
                if not final:
                    for c0, c1 in CH[1:]:
                        acc(em_t, mt, c0, c1, first,
                            stop=(tile_done and c1 == TAPW))
                    if tile_done:
                        rden = ring.tile([128, NPOS], F32, tag="rden",
                                         bufs=2, name=f"rden{mt}")
                        nc.vector.reciprocal_approx_fast(
                            rden[:], dn_ps[mt][:, :NPOS])
                        o_t = ring.tile([128, NPOS], F16, tag="o", bufs=2,
                                        name=f"o{mt}")
                        for c0, c1 in ((0, NPOS // 2), (NPOS // 2, NPOS)):
                            outmul(o_t, rden, mt, c0, c1)
                    return
                # overall-final group (e-chunk already done in emit_ac_e):
                # accumulate m-chunks in slot-halves so the PE work left
                # after the final weight-mult half is small; den completes
                # early -> reciprocal + first out chunk overlap the drain
                def acc_part(c0, c1, s0, ns, stop=False):
                    for sl in range(s0, s0 + ns):
                        nc.tensor.matmul(
                            dn_ps[mt][:, c0:c1], idb[:],
                            em_t[:, sl * TAPW + c0:sl * TAPW + c1],
                            start=False, stop=(stop and sl == s0 + ns - 1))
                acc_part(*CH[1], 0, 4)
                acc_part(*CH[2], 0, 4)
                acc_part(*CH[1], 4, 3)
                rden = ring.tile([128, NPOS], F32, tag="rden", bufs=2,
                                 name=f"rden{mt}")
                nc.vector.reciprocal_approx_fast(rden[:], dn_ps[mt][:, :NPOS])
                o_t = ring.tile([128, NPOS], F16, tag="o", bufs=2,
                                name=f"o{mt}")
                outmul(o_t, rden, mt, 0, 1024 - NPOS)
                acc_part(*CH[2], 4, 3, stop=True)
                outmul(o_t, rden, mt, 1024 - NPOS, NPOS)

            def vcopy(dst, src):
                nc.vector.tensor_copy(dst, src)

            # ---- front: tile1 k/q + first groups overlap remaining GEMMs.
            # tile1's group g reads only kb slab g, so its first score needs
            # just one bias add; tile0's full 7-slab stack builds later.
            kv_gemm("k", 1, k0, nc.scalar.copy)
            bias_adds(1, [0])
            q_gemm(1)
            emit_se(*order[0])            # (0, mt1)
            bias_adds(1, [2, 4])
            kv_gemm("v", 1, v0, nc.scalar.copy)
            nc.gpsimd.dma_start(v1[1][:, :NPAD - 2], v0[1][:, 1:NPAD - 1])
            emit_se(*order[1])            # (2, mt1)
            bias_adds(1, [6, 1, 3, 5])
            kb1_copies(1)
            kv_gemm("k", 0, k0, nc.scalar.copy)
            emit_ac_e(*order[0])
            emit_mu(*order[0])
            bias_adds(0, list(range(K)))
            kb1_copies(0)
            emit_se(*order[2])            # (4, mt1)
            emit_ac_e(*order[1])
            emit_ac(*order[0])
            q_gemm(0)
            kv_gemm("v", 0, v0, nc.scalar.copy)
            nc.gpsimd.dma_start(v1[0][:, :NPAD - 2], v0[0][:, 1:NPAD - 1])
            emit_se(*order[3])            # (6, mt1)
            # ---- steady software pipeline: se three ahead, e-chunk
            # accumulate one ahead of the m-chunk accumulates ----
            for idx in range(1, len(order)):
                if idx + 3 < len(order):
                    emit_se(*order[idx + 3])
                if idx + 1 < len(order):
                    emit_ac_e(*order[idx + 1])
                emit_mu(*order[idx])
                emit_ac(*order[idx])


def _build_graph(repeat=1):
    nc = bacc.Bacc("TRN2", target_bir_lowering=False, debug=False,
                   num_devices=N_CORES)

    GDT = {"f16": F16, "f32": F32}[GEMM_DT]
    dram = (
        nc.declare_dram_parameter("x_slab", [CIN, NPAD], GDT, isOutput=False),
        nc.declare_dram_parameter("wT", [CIN, 3 * COUT], GDT, isOutput=False),
        nc.declare_dram_parameter("cst", [128, 2 * K + 128], F32,
                                  isOutput=False),
        nc.declare_dram_parameter("out", [COUT, NPOS], F16, isOutput=True),
    )

    with tile.TileContext(nc) as tc:
        if repeat > 1:
            with tc.For_i(0, repeat, 1):
                _emit_body(nc, tc, dram)
        else:
            _emit_body(nc, tc, dram)

    nc.compile()
    return nc


def _prep_host(x, w_q, w_k, w_v, rel_h, rel_w):
    gnp = {"f16": np.float16, "f32": np.float32}[GEMM_DT]
    x = np.ascontiguousarray(x, np.float32)
    beta = np.zeros((COUT, K), np.float32)
    beta[:COUT // 2] = np.asarray(rel_h, np.float32).reshape(COUT // 2, K)
    beta[COUT // 2:] = np.asarray(rel_w, np.float32).reshape(COUT // 2, K)
    cst = np.empty((128, 2 * K + 128), np.float32)
    for mt in range(2):
        cst[:, mt * K:(mt + 1) * K] = beta[mt * 128:(mt + 1) * 128]
    cst[:, 2 * K:] = np.eye(128, dtype=np.float32)
    wT = np.concatenate(
        [np.asarray(w_k, np.float32).T, np.asarray(w_q, np.float32).T,
         np.asarray(w_v, np.float32).T], axis=1)
    common = {
        "wT": np.ascontiguousarray(wT, gnp),
        "cst": cst,
    }
    in_maps = []
    for core in range(N_CORES):
        b, r0 = divmod(core, 4)
        r0 *= ROWS
        slab = np.zeros((CIN, SH, SW), np.float32)
        lo, hi = r0 - PAD, r0 + ROWS + PAD
        clo, chi = max(lo, 0), min(hi, H)
        slab[:, clo - lo:chi - lo, PAD:PAD + W] = x[b, :, clo:chi, :]
        in_maps.append({"x_slab": slab.reshape(CIN, NPAD).astype(gnp),
                        **common})
    return in_maps


def kernel(x, w_q, w_k, w_v, rel_h, rel_w):
    if "nc" not in _CACHED:
        _CACHED["nc"] = _build_graph()
    nc = _CACHED["nc"]
    in_maps = _prep_host(x, w_q, w_k, w_v, rel_h, rel_w)
    res = run_bass_kernel_spmd(nc, in_maps, core_ids=list(range(N_CORES)))
    _CACHED["exec_time_ns"] = res.exec_time_ns
    out = np.empty((B, COUT, H, W), np.float32)
    for core in range(N_CORES):
        b, r0 = divmod(core, 4)
        r0 *= ROWS
        out[b, :, r0:r0 + ROWS, :] = \
            res.results[core]["out"].astype(np.float32).reshape(COUT, ROWS, W)
    return out
